# revision 3
# baseline (speedup 1.0000x reference)
"""Trainium2 Bass kernel for the 1x1-conv attention module (fp8 DoubleRow).

Shapes (hardcoded): x (8, 64, 64, 64) fp32, w_qkv (192, 64), b_qkv (192,),
w_out (64, 64), b_out (64,). Data-parallel: one batch element per NeuronCore
(8 cores). Channel-major everywhere (c on partitions, t = h*64+w free); the
reference's view/permute quirk composes to the identity on this layout.

v2 pipeline: the two big GEMMs run in fp8e4m3 with MatmulPerfMode.DoubleRow
(0.5 PE cycles per output column):
  QK^T: lhsT = k-tile [64, 2, 128] (subtile 1 zeroed), rhs = q [64, 2, 512]
  (subtile 1 zeroed) -> scores_T [128 keys, 512 queries] fp32 PSUM in 256
  cycles.  exp() on ScalarE reads 3 PSUM banks per wave and writes fp8
  directly into a per-i-chunk e_all [128, 32, 512] SBUF tile, with the
  softmax scale folded into w_q host-side and a -3.0 bias (cancels in the
  normalization) keeping e^s below fp8e4m3's 240 max.  PV: lhsT = v-pair
  [128, 2, 65] fp8 (rows padded to 80 B -- ldweights needs 16 B-aligned row
  stride; column 64 is ones so the accumulation also yields softmax row
  sums), rhs = e_all pair [128, 2, 512] -> 256 keys contracted per 256-cycle
  matmul.  PE total ~51 us/core; ScalarE exp (~129 us) is the bottleneck,
  so PV defers one wave behind exp (carried across i-chunk boundaries) and
  the per-chunk normalize + output-projection + residual tail is interleaved
  into the next chunk's waves; projections for the next repeat absorb the
  final chunk's tail.  fp8 rounding adds ~2e-3 relative error (gate: 2e-2).
"""

import numpy as np

B, C, HW = 8, 64, 4096
NCORES = 8
IC = 512  # i-chunk (query tokens per block)
NIC = HW // IC  # 8
NJ = HW // 128  # 32 j-tiles of 128 tokens
NJP = NJ // 2  # 16 j-tile pairs (PV DoubleRow)
VP = 80  # v row stride (padded from 65; ldweights wants 16B multiple)

_compiled = None


def _build_bass(repeat=1, do_exp=True, do_av=True, do_norm=True,
                defer_tail=True, wave4=False):
    import concourse.bass as bass
    import concourse.mybir as mybir
    import concourse.tile as tile

    FP = mybir.dt.float32
    FR = mybir.dt.float32r
    F8 = mybir.dt.float8e4
    Exp = mybir.ActivationFunctionType.Exp
    DR = mybir.MatmulPerfMode.DoubleRow

    nc = bass.Bass("TRN2", target_bir_lowering=False, debug=False)

    xa_d = nc.dram_tensor("xa", [C + 1, HW], FP, kind="ExternalInput")
    xb_d = nc.dram_tensor("xb", [C, HW], FP, kind="ExternalInput")
    wq_d = nc.dram_tensor("wq", [C + 1, C], FP, kind="ExternalInput")
    wk_d = nc.dram_tensor("wk", [C + 1, C], FP, kind="ExternalInput")
    wv_d = nc.dram_tensor("wv", [C + 1, C], FP, kind="ExternalInput")
    wo_d = nc.dram_tensor("wo", [C, C], FP, kind="ExternalInput")
    out_d = nc.dram_tensor("out", [C, HW], FP, kind="ExternalOutput")

    with tile.TileContext(nc) as tc:
        with (
            nc.allow_low_precision(reason="fp8 matmul operands (fp32 PSUM accum)"),
            tc.tile_pool(name="singles", bufs=1) as singles,
            tc.tile_pool(name="eall", bufs=2) as epool,
            tc.tile_pool(name="outp", bufs=2) as opool,
            tc.tile_pool(name="sps", bufs=2, space="PSUM") as spool,
            tc.tile_pool(name="ups", bufs=1, space="PSUM") as upool,
            tc.tile_pool(name="rps", bufs=1, space="PSUM") as rpool,
        ):
            # ---- load inputs ----
            xa = singles.tile([C + 1, HW], FP)
            xb = singles.tile([C, HW], FP)
            wq = singles.tile([C + 1, C], FP)
            wk = singles.tile([C + 1, C], FP)
            wv = singles.tile([C + 1, C], FP)
            wo = singles.tile([C, C], FP)
            nc.sync.dma_start(out=xa[:], in_=xa_d[:])
            nc.sync.dma_start(out=xb[:], in_=xb_d[:])
            nc.sync.dma_start(out=wq[:], in_=wq_d[:])
            nc.sync.dma_start(out=wk[:], in_=wk_d[:])
            nc.sync.dma_start(out=wv[:], in_=wv_d[:])
            nc.sync.dma_start(out=wo[:], in_=wo_d[:])

            # fp32r copies of matmul operands (walrus requires matmul inputs
            # produced rounded by an engine op)
            xar = singles.tile([C + 1, HW], FR)
            wqr = singles.tile([C + 1, C], FR)
            wkr = singles.tile([C + 1, C], FR)
            wvr = singles.tile([C + 1, C], FR)
            wor = singles.tile([C, C], FR)
            nc.vector.tensor_copy(xar[:], xa[:])
            nc.vector.tensor_copy(wqr[:], wq[:])
            nc.vector.tensor_copy(wkr[:], wk[:])
            nc.vector.tensor_copy(wvr[:], wv[:])
            nc.vector.tensor_copy(wor[:], wo[:])

            ones32 = singles.tile([128, 1], FP)
            nc.vector.memset(ones32[:], 1.0)
            nbias = singles.tile([128, 1], FP)
            nc.vector.memset(nbias[:], -3.0)
            # preload the exp table while DMAs are in flight
            expwarm = singles.tile([1, 1], FP)
            nc.scalar.activation(expwarm[:], ones32[0:1, :], Exp,
                                 bias=nbias[0:1, :])
            ones_b = singles.tile([1, C], FR)  # K=1 stationary for broadcast
            nc.vector.tensor_copy(
                ones_b[:], ones32[0:1, 0:1].to_broadcast([1, C])
            )

            # fp8 operand tiles (128-partition allocs; QK uses [0:64] slices
            # -- ldweights rejects weight tiles not based at partition 0)
            qf = singles.tile([128, NIC, 2, IC], F8)
            kf = singles.tile([128, NJ, 2, 128], F8)
            vf = singles.tile([128, NJP, 2, VP], F8)
            # zero the spare QK contraction subtile once; ones column for
            # the PV row sums
            nc.vector.memset(qf[0:C, :, 1, :], 0.0)
            nc.vector.memset(kf[0:C, :, 1, :], 0.0)
            nc.vector.memset(vf[:, :, :, C : C + 1], 1.0)

            # unnormalized PV accumulators + row sums + normalized attention
            u_all = singles.tile([C + 1, NIC, IC], FP)
            r_all = singles.tile([1, HW], FR)
            att_all = singles.tile([C, HW], FR)

            if wave4:
                wave_sizes = [4, 3, 4, 3, 4, 3, 4, 3, 4]
            else:
                wave_sizes = [3] * 10 + [2]
            assert sum(wave_sizes) == NJ
            WMAX = max(wave_sizes)

            def emit_q_chunk(n):
                ps3 = spool.tile([128, WMAX, IC], FP, tag="scores", name="sw")
                sl = slice(n * IC, (n + 1) * IC)
                nc.tensor.matmul(ps3[0:C, 0, :], wqr[:], xar[:, sl],
                                 start=True, stop=True)
                nc.vector.tensor_copy(qf[0:C, n, 0, :], ps3[0:C, 0, :])

            def emit_k_chunk(n):
                ps3 = spool.tile([128, WMAX, IC], FP, tag="scores", name="sw")
                sl = slice(n * IC, (n + 1) * IC)
                nc.tensor.matmul(ps3[0:C, 0, :], wkr[:], xar[:, sl],
                                 start=True, stop=True)
                nc.vector.tensor_copy(
                    kf[0:C, 4 * n : 4 * n + 4, 0, :],
                    ps3[0:C, 0, :].rearrange("p (a b) -> p a b", a=4),
                )

            def emit_v_group(g):
                # 8 token-chunks of 128 -> [128, 8, 64] PSUM (one bank)
                pv = rpool.tile([128, 8, C], FP, tag="rsmall", name="pv")
                for t in range(8):
                    jc = g * 8 + t
                    jsl = slice(jc * 128, (jc + 1) * 128)
                    nc.tensor.matmul(pv[:, t, :], xar[:, jsl], wvr[:],
                                     start=True, stop=True)
                nc.vector.tensor_copy(
                    vf[:, 4 * g : 4 * g + 4, :, 0:C],
                    pv[:].rearrange("p (a b) c -> p a b c", a=4),
                )

            def emit_pv(u, ea, jp):
                nc.tensor.matmul(
                    u[:],
                    vf[:, jp, :, 0 : C + 1],
                    ea[:, 2 * jp : 2 * jp + 2, :],
                    start=(jp == 0),
                    stop=(jp == NJP - 1),
                    perf_mode=DR,
                )

            def emit_drain(ic, u):
                nc.vector.tensor_copy(u_all[:, ic, :], u[:])
                if do_norm:
                    nc.vector.reciprocal(
                        r_all[:, ic * IC : (ic + 1) * IC],
                        u_all[C : C + 1, ic, :],
                    )

            def emit_norm(ic):
                isl = slice(ic * IC, (ic + 1) * IC)
                rb = rpool.tile([C, IC], FP, tag="rsmall", name="rb")
                nc.tensor.matmul(rb[:], ones_b[:], r_all[:, isl],
                                 start=True, stop=True)
                nc.vector.tensor_mul(att_all[:, isl], u_all[0:C, ic, :], rb[:])

            def emit_out(ic):
                isl = slice(ic * IC, (ic + 1) * IC)
                p = rpool.tile([C, IC], FP, tag="rsmall", name="po")
                nc.tensor.matmul(p[:], wor[:], att_all[:, isl],
                                 start=True, stop=True)
                o = opool.tile([C, IC], FP, name="o")
                nc.vector.tensor_add(o[:], p[:], xb[:, isl])
                nc.sync.dma_start(out=out_d[:, isl], in_=o[:])

            pending_tail = []
            for _rep in range(repeat):
                # ---- projections (tail of the previous repeat slots in) ----
                for n in range(NIC):
                    emit_k_chunk(n)
                    if n == 2 and pending_tail:
                        emit_norm(pending_tail[0])
                    if n == 5 and pending_tail:
                        emit_out(pending_tail.pop())
                for n in range(NIC):
                    emit_q_chunk(n)
                for g in range(NJ // 8):
                    emit_v_group(g)

                # ---- attention waves ----
                # carry: [ic, u, e_all, next pair] -- PV runs one wave
                # behind exp, carried across i-chunk boundaries
                carry = None

                def pump_pv(carry, ic, jdone):
                    # emit PV pairs whose tiles are all exp'd as of the
                    # previous wave; finish + drain a previous chunk
                    if carry is None:
                        return None
                    cic, cu, cea, cjp = carry
                    lim = NJP if cic != ic else jdone // 2
                    while cjp < lim:
                        emit_pv(cu, cea, cjp)
                        cjp += 1
                    if cic != ic:
                        emit_drain(cic, cu)
                        return None
                    return [cic, cu, cea, cjp]

                for ic in range(NIC):
                    u = upool.tile([C + 1, IC], FP, tag="u")
                    ea = epool.tile([128, NJ, IC], F8, tag="ea")
                    jc = 0
                    for w, ws in enumerate(wave_sizes):
                        s3 = spool.tile([128, WMAX, IC], FP, tag="scores",
                                        name="sw")
                        jbase = jc
                        for t in range(ws):
                            nc.tensor.matmul(
                                s3[:, t, :],
                                kf[0:C, jc, :, :],
                                qf[0:C, ic, :, :],
                                start=True, stop=True,
                                perf_mode=DR,
                            )
                            jc += 1
                        if do_exp:
                            nc.scalar.activation(
                                ea[:, jbase : jbase + ws, :],
                                s3[:, 0:ws, :], Exp, bias=nbias[:],
                            )
                        if do_av:
                            carry = pump_pv(carry, ic, jbase)
                            if carry is None:
                                carry = [ic, u, ea, 0]
                        # interleaved tail of the previous chunk (its drain
                        # completed during this chunk's wave 0)
                        if do_norm and defer_tail and ic >= 1:
                            if w == 2:
                                emit_norm(ic - 1)
                            elif w == 5:
                                emit_out(ic - 1)
                if do_av and carry is not None:
                    carry = pump_pv(carry, -1, 0)
                if do_norm and defer_tail:
                    pending_tail.append(NIC - 1)
                if not (do_norm and defer_tail):
                    for ic in range(NIC):
                        emit_norm(ic)
                    for ic in range(NIC):
                        emit_out(ic)
            for ic in pending_tail:
                emit_norm(ic)
                emit_out(ic)

    _split_matmul_waits(nc, mybir)
    return nc


def _split_matmul_waits(nc, mybir):
    """walrus's codegen only has room for one sync-wait in the engine
    micro-op structs; peel extra waits off onto wait-only EventSemaphore
    instructions on the same engine queue just before.

    First, drop waits that are trivially satisfied: a sem-ge wait on a
    semaphore that is only ever incremented by instructions on this same
    (in-order, FIFO-completing) engine queue is redundant -- by the time
    this instruction dispatches, all its predecessors have completed."""
    skip = (mybir.InstEventSemaphore,)
    sem_engines = {}
    sem_clean = {}
    for bb in nc.main_func.blocks:
        for ins in bb.instructions:
            si = ins.sync_info
            if si is None or not si.on_update:
                continue
            for up in si.on_update:
                sem_engines.setdefault(up.id, set()).add(str(ins.engine))
                # DMA sem increments fire at (async) DMA completion, not
                # at queue progress -- never treat those as queue-ordered
                ok = (
                    up.update_mode == "sem-inc"
                    and up.update_reg is None
                    and "DMA" not in type(ins).__name__
                )
                sem_clean[up.id] = sem_clean.get(up.id, True) and ok

    def is_redundant(ins, wait):
        return (
            wait.wait_mode == "sem-ge-imm"
            and wait.wait_reg is None
            and sem_clean.get(wait.id, False)
            and sem_engines.get(wait.id) == {str(ins.engine)}
        )

    for bb in nc.main_func.blocks:
        for ins in bb.instructions:
            if isinstance(ins, skip):
                continue
            si = ins.sync_info
            if si is not None and si.on_wait and len(si.on_wait) > 1:
                kept = [w for w in si.on_wait if not is_redundant(ins, w)]
                if len(kept) != len(si.on_wait):
                    if not kept:
                        kept = [si.on_wait[-1]]
                    ins.sync_info = mybir.SyncInfo(
                        on_wait=kept, on_update=list(si.on_update or [])
                    )
    for bb in nc.main_func.blocks:
        insts = list(bb.instructions)
        out = []
        changed = False
        for ins in insts:
            if not isinstance(ins, skip):
                si = ins.sync_info
                if si is not None and si.on_wait and len(si.on_wait) > 1:
                    for wi, wait in enumerate(si.on_wait[:-1]):
                        w = mybir.InstEventSemaphore(
                            name=f"{ins.name}_prewait{wi}"
                        )
                        w.engine = ins.engine
                        w.sync_info = mybir.SyncInfo(
                            on_wait=[wait], on_update=[]
                        )
                        out.append(w)
                    ins.sync_info = mybir.SyncInfo(
                        on_wait=[si.on_wait[-1]],
                        on_update=list(si.on_update or []),
                    )
                    changed = True
            out.append(ins)
        if changed:
            bb.instructions = out


def _prep_inputs(x, w_qkv, b_qkv, w_out, b_out):
    """Host-side input prep -> per-core in_maps."""
    x = np.ascontiguousarray(np.asarray(x, dtype=np.float32))
    w_qkv = np.asarray(w_qkv, dtype=np.float32)
    b_qkv = np.asarray(b_qkv, dtype=np.float32)
    w_out = np.asarray(w_out, dtype=np.float32)
    b_out = np.asarray(b_out, dtype=np.float32)

    scale = 1.0 / np.sqrt(np.float32(C))
    wq = np.concatenate([w_qkv[0:C].T, b_qkv[None, 0:C]], axis=0) * scale
    wk = np.concatenate([w_qkv[C : 2 * C].T, b_qkv[None, C : 2 * C]], axis=0)
    wv = np.concatenate([w_qkv[2 * C :].T, b_qkv[None, 2 * C :]], axis=0)
    wo = np.ascontiguousarray(w_out.T)
    wq = np.ascontiguousarray(wq, dtype=np.float32)
    wk = np.ascontiguousarray(wk, dtype=np.float32)
    wv = np.ascontiguousarray(wv, dtype=np.float32)

    ones = np.ones((1, HW), dtype=np.float32)
    in_maps = []
    for b in range(B):
        xcm = x[b].reshape(C, HW)
        xa = np.concatenate([xcm, ones], axis=0)
        xb = xcm + b_out[:, None].astype(np.float32)
        in_maps.append(
            {
                "xa": np.ascontiguousarray(xa),
                "xb": np.ascontiguousarray(xb),
                "wq": wq,
                "wk": wk,
                "wv": wv,
                "wo": wo,
            }
        )
    return in_maps


def _get_compiled():
    global _compiled
    if _compiled is None:
        _compiled = _build_bass()
    return _compiled


def kernel(x, w_qkv, b_qkv, w_out, b_out, _trace=False, _trace_kwargs=None):
    from concourse.bass_utils import run_bass_kernel_spmd

    nc = _get_compiled()
    in_maps = _prep_inputs(x, w_qkv, b_qkv, w_out, b_out)
    res = run_bass_kernel_spmd(
        nc,
        in_maps,
        list(range(NCORES)),
        trace=_trace,
        **(_trace_kwargs or {}),
    )
    out = np.stack([res.results[b]["out"].reshape(C, 64, 64) for b in range(B)])
    if _trace:
        kernel._last_results = res
    return out.astype(np.float32)
